# revision 7
# baseline (speedup 1.0000x reference)
"""GCN layer relu(GCNConv(x, edge_index)) on 8 Trainium2 NeuronCores.

Math (PyG GCNConv with self-loops, symmetric norm):
    deg[v]  = 1 + in-degree(v)
    s       = deg ** -0.5
    out[d]  = relu(s[d] * (sum_{e: dst(e)=d} s[src_e] * (x[src_e] @ W)) + b)
with the self-loop folded in as a regular edge d -> d.

Distribution: destination nodes are sharded 12500/core.  Per core, the host
lays the shard's incoming edges out as a degree-sorted padded ELL table of
"slots" (slot 0 of each node = its self-loop) and ships, for every slot, the
source node's x row in fp8-e3m4 (zero for padding), plus the fp16 norm
scalars s[src] per slot and s[dst] per node (values come from a small
deg**-0.5 table indexed by the integer degrees; all tensor arithmetic stays
on device).

Device pipeline per core:
  stage1  per 128-slot chunk: one matmul, fp8 x-chunk stationary x fp16 W
          moving -> per-slot messages in PSUM [slot, 32].
  scale   DVE: stage = ps * s_src (per-slot broadcast), fp16 SBUF.
  stage2  segment sum on the PE: identity-stationary matmuls accumulate the
          K slot-planes of each node tile into PSUM, batched across runs of
          equal-K tiles (wide strided moving operand).  A 1-contract outer-
          product matmul seeds PSUM with b/s[dst] when b != 0.
  epilog  one ACT pass per tile: relu(s_own * psum) -> out tile.

The xe stream is cut into per-DMA blocks with a ramped schedule (small
blocks at the ends, 2MB in the middle) so the pipeline fills fast and
drains fast; every DMA block is a contiguous HBM region.

Indirect DMA is deliberately avoided: TRN2's dynamic DMA honors only one
runtime offset per partition per instruction, far too slow for 1.7M edge
gathers.  Replicating x per edge costs a larger but perfectly sequential
HBM stream instead; fp8 halves it vs fp16.

Host-side prep is index bookkeeping only (shard, sort, replicate rows, cast,
constant-table lookups of deg**-0.5); all tensor arithmetic happens on
device.
"""

import math
import numpy as np
import ml_dtypes

import concourse.bass as bass
import concourse.bacc as bacc
import concourse.mybir as mybir
import concourse.tile as tile
from concourse import bass_utils

# ---------------------------------------------------------------- config ---
P = 128            # partitions
D_IN = 128
D_OUT = 32
N = 100000         # nodes
E = 1600000        # edges
NCORES = 8

NPC = N // NCORES              # 12500 nodes per core
TPC = math.ceil(NPC / P)       # 98 node tiles per core
NPOS = TPC * P                 # 12544 padded positions per core
NPAD0 = NPOS - NPC             # 44 pad positions (front, degree 0)
NV = NCORES * NPOS             # padded global positions

CBLK = 32                      # slot-columns per matmul/scale sub-block
NTMAX = 16                     # max tiles per stage2 batch (512 moving cols)

F8 = mybir.dt.float8e3
F16 = mybir.dt.float16
F32 = mybir.dt.float32


def block_schedule(totk8):
    """Ramped list of per-DMA column counts summing to totk8 (each a
    multiple of 16, mid-stream blocks 128 cols = 2MB fp8)."""
    sched = []
    rem = totk8
    for c in (16, 16, 32, 64):
        if rem >= c + 128 or rem == c:
            sched.append(c)
            rem -= c
    while rem >= 128:
        sched.append(128)
        rem -= 128
    if rem:
        sched.append(rem)
    assert sum(sched) == totk8
    return sched


# ------------------------------------------------------------- host prep ---
def host_prep(x, edge_index, W, b):
    src = np.asarray(edge_index[0]).astype(np.int64)
    dst = np.asarray(edge_index[1]).astype(np.int64)
    deg = np.bincount(dst, minlength=N).astype(np.int64) + 1   # + self loop

    # Per-core degree sort (ascending); pads sit in front with slot-deg 0.
    node_of_pos = np.full(NV, -1, dtype=np.int64)
    pos_of_node = np.empty(N, dtype=np.int64)
    for c in range(NCORES):
        lo = c * NPC
        order = np.argsort(deg[lo:lo + NPC], kind="stable")
        qs = c * NPOS + NPAD0 + np.arange(NPC)
        node_of_pos[qs] = lo + order
        pos_of_node[lo + order] = qs

    sdeg = np.zeros(NV, dtype=np.int64)
    valid = node_of_pos >= 0
    sdeg[valid] = deg[node_of_pos[valid]]

    # Per-tile slot count K_t, shared across cores (SPMD: one program).
    ktile = sdeg.reshape(NCORES, TPC, P).max(axis=(0, 2))
    ktile = np.maximum(ktile, 1).astype(np.int64)
    offs = np.concatenate([[0], np.cumsum(ktile)]).astype(np.int64)
    totk = int(offs[-1])
    totk8 = (totk + CBLK - 1) // CBLK * CBLK

    # slot source table: src_slot[core][p, c] = source node of that slot
    # (-1 for padding).  Slot offs[t]+0 of node (t,p) is its self loop.
    src_slot = np.full((NCORES, P, totk8), -1, dtype=np.int64)
    vreal = np.nonzero(valid)[0]
    rp = vreal % P
    rt = (vreal % NPOS) // P
    rc = vreal // NPOS
    src_slot[rc, rp, offs[rt]] = node_of_pos[vreal]          # self slots
    key = pos_of_node[dst]
    es = np.argsort(key, kind="stable")
    key_s = key[es]
    src_s = src[es]
    newrun = np.ones(E, dtype=bool)
    newrun[1:] = key_s[1:] != key_s[:-1]
    run_start = np.maximum.accumulate(np.where(newrun, np.arange(E), 0))
    kwith = np.arange(E) - run_start + 1
    ep = key_s % P
    et = (key_s % NPOS) // P
    ec = key_s // NPOS
    src_slot[ec, ep, offs[et] + kwith] = src_s

    # deg**-0.5 constant table (indexed by integer degree; deg 0 -> 0).
    maxdeg = int(deg.max())
    stab = np.zeros(maxdeg + 1, dtype=np.float64)
    stab[1:] = 1.0 / np.sqrt(np.arange(1, maxdeg + 1, dtype=np.float64))
    rtab = np.zeros(maxdeg + 1, dtype=np.float64)
    rtab[1:] = np.sqrt(np.arange(1, maxdeg + 1, dtype=np.float64))

    # xe[core]: ramped contiguous DMA blocks; block i covers sched[i] slot
    # columns, column c*128+p holds x[src_slot[p, c]] (feature on partitions).
    sched = block_schedule(totk8)
    x8 = np.concatenate(
        [np.asarray(x).astype(ml_dtypes.float8_e3m4),
         np.zeros((1, D_IN), ml_dtypes.float8_e3m4)]
    )
    deg_aug = np.concatenate([deg, [0]])
    xe = np.empty((NCORES, P * totk8 * P), dtype=ml_dtypes.float8_e3m4)
    sslot = np.empty((NCORES, P, totk8), dtype=np.float16)
    for c in range(NCORES):
        cols = src_slot[c].T.ravel()                 # j = cc*128 + p
        xc = x8[cols].T                              # [128, totk8*128]
        pos = 0
        col0 = 0
        for ncols in sched:
            blk = xc[:, col0 * P:(col0 + ncols) * P]
            n = blk.size
            xe[c, pos:pos + n] = blk.ravel()
            pos += n
            col0 += ncols
        sslot[c] = stab[deg_aug[src_slot[c]]].astype(np.float16)

    # own-node scales per (p, t): s_own = deg**-0.5 (0 for pads), and
    # rs = deg**0.5 laid out [1, NPOS] for the bias seed outer product.
    sd = sdeg.reshape(NCORES, TPC, P)
    sown = np.empty((NCORES, P, TPC), dtype=np.float32)
    rsrow = np.empty((NCORES, 1, NPOS), dtype=np.float16)
    for c in range(NCORES):
        sown[c] = stab[sd[c]].T.astype(np.float32)
        rsrow[c, 0] = rtab[sd[c]].reshape(NPOS).astype(np.float16)

    w16 = np.asarray(W).astype(np.float16)
    brow = np.asarray(b).astype(np.float16).reshape(1, D_OUT)
    ident = np.eye(P, dtype=np.float16)
    has_bias = bool(np.any(np.asarray(b) != 0))
    return (xe, sslot, sown, rsrow, w16, brow, ident, ktile, offs, totk8,
            node_of_pos, has_bias)


# --------------------------------------------------------------- builder ---
def build_nc(ktile, offs, totk8, has_bias):
    """Build the SPMD bass program for the K-profile of this graph."""
    nc = bacc.Bacc(None, num_devices=NCORES)
    sched = block_schedule(totk8)

    xe = nc.dram_tensor("xe", [P * totk8 * P], F8, kind="ExternalInput")
    sslot = nc.dram_tensor("sslot", [P, totk8], F16, kind="ExternalInput")
    sown = nc.dram_tensor("sown", [P, TPC], F32, kind="ExternalInput")
    rsrow = nc.dram_tensor("rsrow", [1, NPOS], F16, kind="ExternalInput")
    w = nc.dram_tensor("w", [P, D_OUT], F16, kind="ExternalInput")
    brow = nc.dram_tensor("brow", [1, D_OUT], F16, kind="ExternalInput")
    ident = nc.dram_tensor("ident", [P, P], F16, kind="ExternalInput")
    out = nc.dram_tensor("out", [P, TPC * D_OUT], F32, kind="ExternalOutput")

    # stage2 batches: runs of equal-K tiles, at most NTMAX tiles per batch
    kgroups = []
    t0 = 0
    while t0 < TPC:
        t1 = t0 + 1
        while (t1 < TPC and ktile[t1] == ktile[t0]
               and t1 - t0 < NTMAX):
            t1 += 1
        kgroups.append((t0, t1, int(ktile[t0])))
        t0 = t1

    with tile.TileContext(nc) as tc:
        with (
            tc.tile_pool(name="const", bufs=1) as cpool,
            tc.tile_pool(name="stage", bufs=1) as spool,
            tc.tile_pool(name="xin", bufs=3) as xpool,
            tc.tile_pool(name="ps1", bufs=2, space="PSUM") as ps1_pool,
            tc.tile_pool(name="ps2", bufs=2, space="PSUM") as ps2_pool,
        ):
            w_sb = cpool.tile([P, D_OUT], F16)
            id_sb = cpool.tile([P, P], F16)
            b_sb = cpool.tile([1, D_OUT], F16)
            rs_sb = cpool.tile([1, NPOS], F16)
            sslot_sb = cpool.tile([P, totk8], F16)
            sown_sb = cpool.tile([P, TPC], F32)
            tbuf = cpool.tile([P, TPC * D_OUT], F32)
            stage = spool.tile([P, totk8 * D_OUT], F16)

            nc.sync.dma_start(out=w_sb[:], in_=w[:, :])

            stage_row = totk8 * D_OUT

            def emit_stage2(t0, t1, K):
                nt = t1 - t0
                acc = ps2_pool.tile([P, NTMAX * D_OUT], F32, tag="acc")
                first = True
                if has_bias:
                    for ti in range(nt):
                        nc.tensor.matmul(
                            out=acc[:, ti * D_OUT:(ti + 1) * D_OUT],
                            lhsT=rs_sb[0:1, (t0 + ti) * P:(t0 + ti + 1) * P],
                            rhs=b_sb[0:1, :],
                            start=True, stop=False,
                            skip_group_check=True,
                        )
                    first = False
                for k in range(K):
                    rhs = bass.AP(
                        stage[:].tensor,
                        stage[:].offset + (int(offs[t0]) + k) * D_OUT,
                        [[stage_row, P], [K * D_OUT, nt], [1, D_OUT]],
                    )
                    nc.tensor.matmul(
                        out=acc[:, 0:nt * D_OUT],
                        lhsT=id_sb[:],
                        rhs=rhs,
                        start=first, stop=(k == K - 1),
                        skip_group_check=True,
                    )
                    first = False
                # epilogue: relu(s_own * acc) per tile (ACT, psum -> sbuf)
                for ti in range(nt):
                    t = t0 + ti
                    nc.scalar.activation(
                        out=tbuf[:, t * D_OUT:(t + 1) * D_OUT],
                        in_=acc[:, ti * D_OUT:(ti + 1) * D_OUT],
                        func=mybir.ActivationFunctionType.Relu,
                        scale=sown_sb[:, t:t + 1],
                    )

            gi = 0          # next kgroup to emit
            tout = [0]      # tiles whose output DMA has been issued

            def flush_out(upto_tile):
                t0o = tout[0]
                if upto_tile > t0o:
                    nc.scalar.dma_start(
                        out=out[:, t0o * D_OUT:upto_tile * D_OUT],
                        in_=tbuf[:, t0o * D_OUT:upto_tile * D_OUT],
                    )
                    tout[0] = upto_tile

            col0 = 0
            pos0 = 0
            for bi, ncols in enumerate(sched):
                xblk = xpool.tile([P, 128 * P], F8, tag="xblk")
                nc.sync.dma_start(
                    out=xblk[:, 0:ncols * P],
                    in_=xe[pos0:pos0 + P * ncols * P].rearrange(
                        "(p c) -> p c", c=ncols * P),
                )
                if bi == 0:
                    # consts go on the scalar-engine HWDGE ring so the sync
                    # ring stays a pure xe read stream
                    nc.scalar.dma_start(out=id_sb[:], in_=ident[:, :])
                    nc.scalar.dma_start(out=b_sb[:], in_=brow[:, :])
                    nc.scalar.dma_start(out=rs_sb[:], in_=rsrow[:, :])
                    nc.scalar.dma_start(out=sslot_sb[:], in_=sslot[:, :])
                    nc.scalar.dma_start(out=sown_sb[:], in_=sown[:, :])
                for sub in range(0, ncols, CBLK):
                    nsub = min(CBLK, ncols - sub)
                    c0 = col0 + sub
                    ps = ps1_pool.tile([P, CBLK * D_OUT], F32, tag="ps")
                    for j in range(nsub):
                        nc.tensor.matmul(
                            out=ps[:, j * D_OUT:(j + 1) * D_OUT],
                            lhsT=xblk[:, (sub + j) * P:(sub + j + 1) * P],
                            rhs=w_sb[:],
                            start=True, stop=True,
                        )
                    nc.vector.tensor_tensor(
                        out=stage[:, c0 * D_OUT:(c0 + nsub) * D_OUT]
                        .rearrange("p (c f) -> p c f", f=D_OUT),
                        in0=ps[:, 0:nsub * D_OUT]
                        .rearrange("p (c f) -> p c f", f=D_OUT),
                        in1=sslot_sb[:, c0:c0 + nsub].to_broadcast(
                            [P, nsub, D_OUT]
                        ),
                        op=mybir.AluOpType.mult,
                    )
                    # emit stage2 for kgroups fully covered by scaled cols
                    done = c0 + nsub
                    while gi < len(kgroups) and \
                            int(offs[kgroups[gi][1]]) <= done:
                        emit_stage2(*kgroups[gi])
                        gi += 1
                        if kgroups[gi - 1][1] - tout[0] >= 12:
                            flush_out(kgroups[gi - 1][1])
                col0 += ncols
                pos0 += P * ncols * P
            while gi < len(kgroups):
                emit_stage2(*kgroups[gi])
                gi += 1
            flush_out(TPC)

    nc.finalize()
    return nc


# ---------------------------------------------------------------- runner ---
def _run(inputs, trace=False):
    (xe, sslot, sown, rsrow, w16, brow, ident, ktile, offs, totk8,
     node_of_pos, has_bias) = host_prep(
        inputs["x"], inputs["edge_index"], inputs["W"], inputs["b"]
    )
    nc = build_nc(ktile, offs, totk8, has_bias)
    in_maps = [
        {"xe": xe[c], "sslot": sslot[c], "sown": sown[c], "rsrow": rsrow[c],
         "w": w16, "brow": brow, "ident": ident}
        for c in range(NCORES)
    ]
    res = bass_utils.run_bass_kernel_spmd(
        nc, in_maps, core_ids=list(range(NCORES)), trace=trace
    )
    full = np.empty((N, D_OUT), dtype=np.float32)
    for c in range(NCORES):
        oc = res.results[c]["out"].reshape(P, TPC, D_OUT)
        block = oc.transpose(1, 0, 2).reshape(NPOS, D_OUT)
        nid = node_of_pos[c * NPOS:(c + 1) * NPOS]
        m = nid >= 0
        full[nid[m]] = block[m]
    return full, res


def kernel(**inputs) -> np.ndarray:
    full, _ = _run(inputs, trace=False)
    return full


# revision 8
# speedup vs baseline: 1.1279x; 1.1279x over previous
"""GCN layer relu(GCNConv(x, edge_index)) on 8 Trainium2 NeuronCores.

Math (PyG GCNConv with self-loops, symmetric norm):
    deg[v]  = 1 + in-degree(v)
    s       = deg ** -0.5
    out[d]  = relu(s[d] * (sum_{e: dst(e)=d} s[src_e] * (x[src_e] @ W)) + b)
with the self-loop folded in as a regular edge d -> d.

Distribution: destination nodes are sharded 12500/core.  Per core, the host
lays the shard's incoming edges out as a degree-sorted padded ELL table of
"slots" (slot 0 of each node = its self-loop) and ships, for every slot, the
source node's x row in fp8-e3m4 (zero for padding), plus the fp16 norm
scalars s[src] per slot and s[dst] per node (values come from a small
deg**-0.5 table indexed by the integer degrees; all tensor arithmetic stays
on device).

Device pipeline per core:
  stage1  per 128-slot chunk: one matmul, fp8 x-chunk stationary x fp16 W
          moving -> per-slot messages in PSUM [slot, 32].
  scale   DVE: stage = ps * s_src (per-slot broadcast), fp16 SBUF.
  stage2  segment sum on the PE: identity-stationary matmuls accumulate the
          K slot-planes of each node tile into PSUM, batched across runs of
          equal-K tiles (wide strided moving operand).  A 1-contract outer-
          product matmul seeds PSUM with b/s[dst] when b != 0.
  epilog  one ACT pass per tile: relu(s_own * psum) -> out tile.

The xe stream is cut into per-DMA blocks with a ramped schedule (small
blocks at the ends, 2MB in the middle) so the pipeline fills fast and
drains fast; every DMA block is a contiguous HBM region.

Indirect DMA is deliberately avoided: TRN2's dynamic DMA honors only one
runtime offset per partition per instruction, far too slow for 1.7M edge
gathers.  Replicating x per edge costs a larger but perfectly sequential
HBM stream instead; fp8 halves it vs fp16.

Host-side prep is index bookkeeping only (shard, sort, replicate rows, cast,
constant-table lookups of deg**-0.5); all tensor arithmetic happens on
device.
"""

import math
import numpy as np
import ml_dtypes

import concourse.bass as bass
import concourse.bacc as bacc
import concourse.mybir as mybir
import concourse.tile as tile
from concourse import bass_utils

# ---------------------------------------------------------------- config ---
P = 128            # partitions
D_IN = 128
D_OUT = 32
N = 100000         # nodes
E = 1600000        # edges
NCORES = 8

NPC = N // NCORES              # 12500 nodes per core
TPC = math.ceil(NPC / P)       # 98 node tiles per core
NPOS = TPC * P                 # 12544 padded positions per core
NPAD0 = NPOS - NPC             # 44 pad positions (front, degree 0)
NV = NCORES * NPOS             # padded global positions

CBLK = 32                      # slot-columns per matmul/scale sub-block
NTMAX = 16                     # max tiles per stage2 batch (512 moving cols)

F8 = mybir.dt.float8e3
F16 = mybir.dt.float16
F32 = mybir.dt.float32


def block_schedule(totk8):
    """Ramped list of per-DMA column counts summing to totk8 (each a
    multiple of 16, mid-stream blocks 128 cols = 2MB fp8)."""
    sched = []
    rem = totk8
    for c in (16, 16, 32, 64):
        if rem >= c + 128 or rem == c:
            sched.append(c)
            rem -= c
    while rem >= 128:
        sched.append(128)
        rem -= 128
    if rem:
        sched.append(rem)
    assert sum(sched) == totk8
    return sched


# ------------------------------------------------------------- host prep ---
def host_prep(x, edge_index, W, b):
    src = np.asarray(edge_index[0]).astype(np.int64)
    dst = np.asarray(edge_index[1]).astype(np.int64)
    deg = np.bincount(dst, minlength=N).astype(np.int64) + 1   # + self loop

    # Per-core degree sort (ascending); pads sit in front with slot-deg 0.
    node_of_pos = np.full(NV, -1, dtype=np.int64)
    pos_of_node = np.empty(N, dtype=np.int64)
    for c in range(NCORES):
        lo = c * NPC
        order = np.argsort(deg[lo:lo + NPC], kind="stable")
        qs = c * NPOS + NPAD0 + np.arange(NPC)
        node_of_pos[qs] = lo + order
        pos_of_node[lo + order] = qs

    sdeg = np.zeros(NV, dtype=np.int64)
    valid = node_of_pos >= 0
    sdeg[valid] = deg[node_of_pos[valid]]

    # Per-tile slot count K_t, shared across cores (SPMD: one program).
    ktile = sdeg.reshape(NCORES, TPC, P).max(axis=(0, 2))
    ktile = np.maximum(ktile, 1).astype(np.int64)
    offs = np.concatenate([[0], np.cumsum(ktile)]).astype(np.int64)
    totk = int(offs[-1])
    totk8 = (totk + CBLK - 1) // CBLK * CBLK

    # slot source table: src_slot[core][p, c] = source node of that slot
    # (-1 for padding).  Slot offs[t]+0 of node (t,p) is its self loop.
    src_slot = np.full((NCORES, P, totk8), -1, dtype=np.int64)
    vreal = np.nonzero(valid)[0]
    rp = vreal % P
    rt = (vreal % NPOS) // P
    rc = vreal // NPOS
    src_slot[rc, rp, offs[rt]] = node_of_pos[vreal]          # self slots
    key = pos_of_node[dst]
    es = np.argsort(key, kind="stable")
    key_s = key[es]
    src_s = src[es]
    newrun = np.ones(E, dtype=bool)
    newrun[1:] = key_s[1:] != key_s[:-1]
    run_start = np.maximum.accumulate(np.where(newrun, np.arange(E), 0))
    kwith = np.arange(E) - run_start + 1
    ep = key_s % P
    et = (key_s % NPOS) // P
    ec = key_s // NPOS
    src_slot[ec, ep, offs[et] + kwith] = src_s

    # deg**-0.5 constant table (indexed by integer degree; deg 0 -> 0).
    maxdeg = int(deg.max())
    stab = np.zeros(maxdeg + 1, dtype=np.float64)
    stab[1:] = 1.0 / np.sqrt(np.arange(1, maxdeg + 1, dtype=np.float64))
    rtab = np.zeros(maxdeg + 1, dtype=np.float64)
    rtab[1:] = np.sqrt(np.arange(1, maxdeg + 1, dtype=np.float64))

    # xe[core]: ramped contiguous DMA blocks; block i covers sched[i] slot
    # columns, column c*128+p holds x[src_slot[p, c]] (feature on partitions).
    sched = block_schedule(totk8)
    x8 = np.concatenate(
        [np.asarray(x).astype(ml_dtypes.float8_e3m4),
         np.zeros((1, D_IN), ml_dtypes.float8_e3m4)]
    )
    deg_aug = np.concatenate([deg, [0]])
    xe = np.empty((NCORES, P * totk8 * P), dtype=ml_dtypes.float8_e3m4)
    sslot = np.empty((NCORES, P, totk8), dtype=np.float16)
    for c in range(NCORES):
        cols = src_slot[c].T.ravel()                 # j = cc*128 + p
        xc = x8[cols].T                              # [128, totk8*128]
        pos = 0
        col0 = 0
        for ncols in sched:
            blk = xc[:, col0 * P:(col0 + ncols) * P]
            n = blk.size
            xe[c, pos:pos + n] = blk.ravel()
            pos += n
            col0 += ncols
        sslot[c] = stab[deg_aug[src_slot[c]]].astype(np.float16)

    # own-node scales per (p, t): s_own = deg**-0.5 (0 for pads), and
    # rs = deg**0.5 laid out [1, NPOS] for the bias seed outer product.
    sd = sdeg.reshape(NCORES, TPC, P)
    sown = np.empty((NCORES, P, TPC), dtype=np.float32)
    rsrow = np.empty((NCORES, 1, NPOS), dtype=np.float16)
    for c in range(NCORES):
        sown[c] = stab[sd[c]].T.astype(np.float32)
        rsrow[c, 0] = rtab[sd[c]].reshape(NPOS).astype(np.float16)

    w16 = np.asarray(W).astype(np.float16)
    brow = np.asarray(b).astype(np.float16).reshape(1, D_OUT)
    ident = np.eye(P, dtype=np.float16)
    has_bias = bool(np.any(np.asarray(b) != 0))
    return (xe, sslot, sown, rsrow, w16, brow, ident, ktile, offs, totk8,
            node_of_pos, has_bias)


# --------------------------------------------------------------- builder ---
def build_nc(ktile, offs, totk8, has_bias):
    """Build the SPMD bass program for the K-profile of this graph."""
    nc = bacc.Bacc(None, num_devices=NCORES)
    sched = block_schedule(totk8)

    xe = nc.dram_tensor("xe", [P * totk8 * P], F8, kind="ExternalInput")
    sslot = nc.dram_tensor("sslot", [P, totk8], F16, kind="ExternalInput")
    sown = nc.dram_tensor("sown", [P, TPC], F32, kind="ExternalInput")
    rsrow = nc.dram_tensor("rsrow", [1, NPOS], F16, kind="ExternalInput")
    w = nc.dram_tensor("w", [P, D_OUT], F16, kind="ExternalInput")
    brow = nc.dram_tensor("brow", [1, D_OUT], F16, kind="ExternalInput")
    ident = nc.dram_tensor("ident", [P, P], F16, kind="ExternalInput")
    out = nc.dram_tensor("out", [P, TPC * D_OUT], F32, kind="ExternalOutput")

    # stage2 batches: runs of equal-K tiles, at most NTMAX tiles per batch
    kgroups = []
    t0 = 0
    while t0 < TPC:
        t1 = t0 + 1
        while (t1 < TPC and ktile[t1] == ktile[t0]
               and t1 - t0 < NTMAX):
            t1 += 1
        kgroups.append((t0, t1, int(ktile[t0])))
        t0 = t1

    with tile.TileContext(nc) as tc:
        with (
            tc.tile_pool(name="const", bufs=1) as cpool,
            tc.tile_pool(name="stage", bufs=1) as spool,
            tc.tile_pool(name="xin", bufs=3) as xpool,
            tc.tile_pool(name="ps1", bufs=2, space="PSUM") as ps1_pool,
            tc.tile_pool(name="ps2", bufs=2, space="PSUM") as ps2_pool,
        ):
            w_sb = cpool.tile([P, D_OUT], F16)
            id_sb = cpool.tile([P, P], F16)
            b_sb = cpool.tile([1, D_OUT], F16)
            rs_sb = cpool.tile([1, NPOS], F16)
            sslot_sb = cpool.tile([P, totk8], F16)
            sown_sb = cpool.tile([P, TPC], F32)
            tbuf = cpool.tile([P, TPC * D_OUT], F32)
            stage = spool.tile([P, totk8 * D_OUT], F16)

            nc.sync.dma_start(out=w_sb[:], in_=w[:, :])

            stage_row = totk8 * D_OUT

            def emit_stage2(t0, t1, K):
                nt = t1 - t0
                acc = ps2_pool.tile([P, NTMAX * D_OUT], F32, tag="acc")
                first = True
                if has_bias:
                    for ti in range(nt):
                        nc.tensor.matmul(
                            out=acc[:, ti * D_OUT:(ti + 1) * D_OUT],
                            lhsT=rs_sb[0:1, (t0 + ti) * P:(t0 + ti + 1) * P],
                            rhs=b_sb[0:1, :],
                            start=True, stop=False,
                            skip_group_check=True,
                        )
                    first = False
                for k in range(K):
                    rhs = bass.AP(
                        stage[:].tensor,
                        stage[:].offset + (int(offs[t0]) + k) * D_OUT,
                        [[stage_row, P], [K * D_OUT, nt], [1, D_OUT]],
                    )
                    nc.tensor.matmul(
                        out=acc[:, 0:nt * D_OUT],
                        lhsT=id_sb[:],
                        rhs=rhs,
                        start=first, stop=(k == K - 1),
                        skip_group_check=True,
                    )
                    first = False
                # epilogue: relu(s_own * acc) per tile (ACT, psum -> sbuf)
                for ti in range(nt):
                    t = t0 + ti
                    nc.scalar.activation(
                        out=tbuf[:, t * D_OUT:(t + 1) * D_OUT],
                        in_=acc[:, ti * D_OUT:(ti + 1) * D_OUT],
                        func=mybir.ActivationFunctionType.Relu,
                        scale=sown_sb[:, t:t + 1],
                    )

            gi = 0          # next kgroup to emit
            tout = [0]      # tiles whose output DMA has been issued

            def flush_out(upto_tile):
                t0o = tout[0]
                if upto_tile > t0o:
                    nc.sync.dma_start(
                        out=out[:, t0o * D_OUT:upto_tile * D_OUT],
                        in_=tbuf[:, t0o * D_OUT:upto_tile * D_OUT],
                    )
                    tout[0] = upto_tile

            col0 = 0
            pos0 = 0
            for bi, ncols in enumerate(sched):
                xblk = xpool.tile([P, 128 * P], F8, tag="xblk")
                nc.sync.dma_start(
                    out=xblk[:, 0:ncols * P],
                    in_=xe[pos0:pos0 + P * ncols * P].rearrange(
                        "(p c) -> p c", c=ncols * P),
                )
                if bi == 0:
                    # small consts ride behind the first (small) xe block
                    nc.sync.dma_start(out=id_sb[:], in_=ident[:, :])
                    nc.sync.dma_start(out=b_sb[:], in_=brow[:, :])
                    nc.sync.dma_start(out=rs_sb[:], in_=rsrow[:, :])
                    nc.sync.dma_start(out=sslot_sb[:], in_=sslot[:, :])
                    nc.sync.dma_start(out=sown_sb[:], in_=sown[:, :])
                for sub in range(0, ncols, CBLK):
                    nsub = min(CBLK, ncols - sub)
                    c0 = col0 + sub
                    ps = ps1_pool.tile([P, CBLK * D_OUT], F32, tag="ps")
                    for j in range(nsub):
                        nc.tensor.matmul(
                            out=ps[:, j * D_OUT:(j + 1) * D_OUT],
                            lhsT=xblk[:, (sub + j) * P:(sub + j + 1) * P],
                            rhs=w_sb[:],
                            start=True, stop=True,
                        )
                    nc.vector.tensor_tensor(
                        out=stage[:, c0 * D_OUT:(c0 + nsub) * D_OUT]
                        .rearrange("p (c f) -> p c f", f=D_OUT),
                        in0=ps[:, 0:nsub * D_OUT]
                        .rearrange("p (c f) -> p c f", f=D_OUT),
                        in1=sslot_sb[:, c0:c0 + nsub].to_broadcast(
                            [P, nsub, D_OUT]
                        ),
                        op=mybir.AluOpType.mult,
                    )
                    # emit stage2 for kgroups fully covered by scaled cols
                    done = c0 + nsub
                    while gi < len(kgroups) and \
                            int(offs[kgroups[gi][1]]) <= done:
                        emit_stage2(*kgroups[gi])
                        gi += 1
                        if tout[0] == 0 and kgroups[gi - 1][1] >= 72:
                            flush_out(kgroups[gi - 1][1])
                col0 += ncols
                pos0 += P * ncols * P
            while gi < len(kgroups):
                emit_stage2(*kgroups[gi])
                gi += 1
            flush_out(TPC)

    nc.finalize()
    return nc


# ---------------------------------------------------------------- runner ---
def _run(inputs, trace=False):
    (xe, sslot, sown, rsrow, w16, brow, ident, ktile, offs, totk8,
     node_of_pos, has_bias) = host_prep(
        inputs["x"], inputs["edge_index"], inputs["W"], inputs["b"]
    )
    nc = build_nc(ktile, offs, totk8, has_bias)
    in_maps = [
        {"xe": xe[c], "sslot": sslot[c], "sown": sown[c], "rsrow": rsrow[c],
         "w": w16, "brow": brow, "ident": ident}
        for c in range(NCORES)
    ]
    res = bass_utils.run_bass_kernel_spmd(
        nc, in_maps, core_ids=list(range(NCORES)), trace=trace
    )
    full = np.empty((N, D_OUT), dtype=np.float32)
    for c in range(NCORES):
        oc = res.results[c]["out"].reshape(P, TPC, D_OUT)
        block = oc.transpose(1, 0, 2).reshape(NPOS, D_OUT)
        nid = node_of_pos[c * NPOS:(c + 1) * NPOS]
        m = nid >= 0
        full[nid[m]] = block[m]
    return full, res


def kernel(**inputs) -> np.ndarray:
    full, _ = _run(inputs, trace=False)
    return full
